# revision 18
# baseline (speedup 1.0000x reference)
"""Trainium2 Bass kernel: separable 25-tap Gaussian blur (sigma=4) on
[1, 3, 4096, 4096] f32 with edge-replicate padding.

reference computes  blur(img/img.max()) * img.max();  conv is linear, so this
equals blur(img) up to f32 rounding -- the global max is skipped.

Scheme (per core, H sharded 8 ways into 512-row slabs + 12-row halos):
  * host: edge-pad to [3, 4120, 4120], scale x8, quantize fp8-e3m4 (4-bit
    mantissa; x8 keeps all pixel values in e3m4 normal range).  Slice
    536-row slabs per core.  Input DMA is 1 B/px.
  * vertical pass:  fused conv+transpose matmuls. For each 128-wide w-slice j,
    out_V[w, h_out 0..511] = sum_t  X_t[:, wsl].T @ M_t   (PSUM accumulate
    over 5 input row-tiles t with banded fp16 constant matrices M_t; the
    fp8 data is the stationary operand, the fp16 band matrix streams).
    Result Ys_j = [w=128 partitions, h=512] fp16, value = 8x true.
  * horizontal pass: identical structure on Ys (contraction now over w),
    which transposes back: out2 = [h=128, w_out] natural layout.  The
    horizontal band matrices carry a 1/8 factor to undo the input scale.
  * PSUM evacuated by copies rotated across ACT / Pool / DVE so no single
    engine becomes the bottleneck; DMA out fp16.
"""

import json
import sys

import ml_dtypes
import numpy as np

SIGMA = 4.0
HALF = 12
KSZ = 25
H, W, C = 4096, 4096, 3
N_CORES = 8
SLAB = H // N_CORES          # 512 output rows per core
PAD_W = W + 2 * HALF         # 4120
IN_ROWS = SLAB + 2 * HALF    # 536 input rows per core
N_WTILES = 33                # ceil(4120 / 128); last tile 24 wide
WINDOWS = [(0, 128), (104, 256), (232, 384), (360, 512), (488, 512)]
# vertical input row-tiles at stride 104 (rows 104t..104t+127): every output
# row's 25 taps live in exactly one tile, so the windows are disjoint
WINDOWS_V = [(0, 104), (104, 208), (208, 312), (312, 416), (416, 512)]
KP_V = [128, 128, 128, 128, 120]
IN_SCALE = 1.0               # fp16 input needs no scaling
E3 = ml_dtypes.float8_e3m4
OUT_DT_NP = np.float16       # output staged in fp16, upcast on host

_PATCHED = False
_NC_CACHE = {}


def _patch_bass_for_this_walrus():
    """This container's walrus encodes at most ONE inline sem wait per
    instruction ("Too many sync wait commands" otherwise).  Tile freely puts
    several waits on one instruction, so rewrite the BIR JSON at serialization
    time: hoist every multi-wait into standalone EventSemaphore instructions
    (the encoding `wait_ge` uses, which this walrus accepts) placed just
    before the instruction on the same engine queue."""
    global _PATCHED
    if _PATCHED:
        return
    import concourse.bass as bass

    orig = bass.Bass.to_json_bytes

    def _split_multi_waits(self):
        raw = orig(self)
        bir = json.loads(raw)
        ctr = 0
        changed = False
        for fn in bir.get("functions", []):
            for blk in fn.get("blocks", []):
                insts = blk.get("instructions")
                if not insts:
                    continue
                new = []
                for ins in insts:
                    si = ins.get("sync_info")
                    waits = (si or {}).get("on_wait") or []
                    if len(waits) > 1:
                        changed = True
                        for w in waits:
                            ctr += 1
                            ev = {
                                "engine": ins["engine"],
                                "ins": [],
                                "outs": [],
                                "name": f"mwsplit_{ctr}_{ins.get('name', '')}",
                                "opcode": "EventSemaphore",
                                "sync_info": {"on_update": [], "on_wait": [w]},
                            }
                            if "debug" in ins:
                                ev["debug"] = ins["debug"]
                            new.append(ev)
                        si["on_wait"] = []
                    new.append(ins)
                blk["instructions"] = new
        if not changed:
            return raw
        return json.dumps(bir).encode()

    bass.Bass.to_json_bytes = _split_multi_waits
    _PATCHED = True


def _gauss_1d():
    x = np.arange(-HALF, HALF + 1, dtype=np.float64)
    k = np.exp(-0.5 * (x / SIGMA) ** 2)
    return k / k.sum()


def _band_v(dtype=np.float16):
    """[128, 104] band: out col n (window-local) from input row p = n..n+24."""
    k = _gauss_1d()
    bv = np.zeros((128, 104), np.float64)
    for p in range(128):
        for n in range(max(0, p - 24), min(104, p + 1)):
            bv[p, n] = k[p - n]
    return bv.astype(dtype)


def _band_matrices(scale=1.0, dtype=np.float16):
    k = _gauss_1d() * scale
    mf = np.zeros((128, 128), np.float64)
    for p in range(128):
        for n in range(max(0, p - 24), p + 1):
            mf[p, n] = k[p - n]
    mm = np.zeros((128, 152), np.float64)
    for p in range(128):
        for n in range(p, min(152, p + 25)):
            mm[p, n] = k[p - n + 24]
    ml = np.zeros((24, 24), np.float64)
    for p in range(24):
        for n in range(p, 24):
            ml[p, n] = k[p - n + 24]
    return mf.astype(dtype), mm.astype(dtype), ml.astype(dtype)


def _build_nc():
    """Build the per-core SPMD Bass program (all 8 cores run the same code on
    different slabs)."""
    _patch_bass_for_this_walrus()
    import concourse.bass as bass
    import concourse.tile as tile
    from concourse import mybir
    from contextlib import ExitStack

    f8 = mybir.dt.float8e3
    f16 = mybir.dt.float16
    f32 = mybir.dt.float32
    out_dt = f16 if OUT_DT_NP == np.float16 else f32

    # band matrices; the horizontal set carries 1/IN_SCALE.  Packed into one
    # [128, 608] fp16 block (cols: mf 128 | mm 152 | ml 24 | x2) so startup
    # is a single small DMA.
    bv_np = _band_v(np.float16)
    mfh_np, mmh_np, mlh_np = _band_matrices(1.0 / IN_SCALE, np.float16)
    packed = np.zeros((128, 408), np.float16)
    packed[:, 0:104] = bv_np
    packed[:, 104:232] = mfh_np
    packed[:, 232:384] = mmh_np
    packed[0:24, 384:408] = mlh_np

    nc = bass.Bass()
    WSPL = 2176                  # w split point for the input DMA halves
    x1a = nc.declare_dram_parameter("x1a", [C, 128, 5, WSPL], f16, isOutput=False)
    x1b = nc.declare_dram_parameter(
        "x1b", [C, 128, 5, PAD_W - WSPL], f16, isOutput=False
    )
    y = nc.declare_dram_parameter("y", [C, 2, 128, 2, W], out_dt, isOutput=True)
    packed_d = nc.inline_tensor(packed, name="bands")

    with tile.TileContext(nc) as tc, ExitStack() as ctx:
        consts = ctx.enter_context(tc.tile_pool(name="consts", bufs=1))
        xpool = ctx.enter_context(tc.tile_pool(name="xp", bufs=2))
        yspool = ctx.enter_context(tc.tile_pool(name="ys", bufs=2))
        opool = ctx.enter_context(tc.tile_pool(name="ostage", bufs=2))
        psv = ctx.enter_context(tc.tile_pool(name="psv", bufs=2, space="PSUM"))
        psh = ctx.enter_context(tc.tile_pool(name="psh", bufs=2, space="PSUM"))

        bands = consts.tile([128, 408], f16)
        nc.sync.dma_start(bands[:], packed_d[:])
        bv = bands[:, 0:104]
        mats_h = [bands[:, 104:232], bands[:, 232:384], bands[:, 232:384],
                  bands[:, 232:384], bands[0:24, 384:408]]

        # pre-warm the tensor engine's clock governor while the first
        # channel's input DMA is in flight: harmless matmuls on the const tile
        wv = psv.tile([128, 1024], f32, name="pv")
        for _ in range(150):
            nc.tensor.matmul(
                out=wv[0:104, 0:104], lhsT=bands[:, 0:104], rhs=bands[:, 0:104],
                start=True, stop=True,
            )

        for c in range(C):
            xt = xpool.tile([128, 5, PAD_W], f16)
            # p-major packed contiguous descriptors; w-split so the first
            # half of the vertical pass can start before the rest lands
            nc.sync.dma_start(xt[:, :, 0:WSPL], x1a[c])
            nc.sync.dma_start(xt[:, :, WSPL:PAD_W], x1b[c])

            ys = yspool.tile([128, N_WTILES, 512], f16)

            # vertical pass (conv over h, output transposed to [w, h]);
            # two w-slices share a 2-bank PSUM tile -> 1024-wide DVE evacs
            for jp in range((N_WTILES + 1) // 2):
                js = [2 * jp] + ([2 * jp + 1] if 2 * jp + 1 < N_WTILES else [])
                pv = psv.tile([128, 1024], f32, name="pv")
                for ji, j in enumerate(js):
                    m = 128 if j < N_WTILES - 1 else PAD_W - 128 * (N_WTILES - 1)
                    for t in range(5):
                        n0, n1 = WINDOWS_V[t]
                        kp = KP_V[t]
                        nc.tensor.matmul(
                            out=pv[0:m, 512 * ji + n0 : 512 * ji + n1],
                            lhsT=xt[0:kp, t, 128 * j : 128 * j + m],
                            rhs=bv[0:kp, 0 : n1 - n0],
                            start=(t == 0),
                            stop=(t == 4),
                        )
                vcopy = nc.vector.tensor_copy
                if len(js) == 2:
                    vcopy(ys[:, js[0] : js[0] + 2, :], pv[:, :])
                else:
                    m = PAD_W - 128 * (N_WTILES - 1)
                    vcopy(ys[0:m, js[0], :], pv[0:m, 0:512])

            # horizontal pass (conv over w, transposes back to [h, w]);
            # two h-blocks share one staging tile so each output DMA
            # descriptor covers two DRAM rows (16 KB contiguous)
            for b2 in range(2):
                ot = opool.tile([128, 2, W], out_dt)
                for bi in range(2):
                    b = 2 * b2 + bi
                    for qp in range(W // 1024):
                        ph = psh.tile([128, 1024], f32)
                        for qi in range(2):
                            q = 2 * qp + qi
                            for t in range(5):
                                j = 4 * q + t
                                n0, n1 = WINDOWS[t]
                                kp = 128 if (t < 4 and j < N_WTILES - 1) else 24
                                nc.tensor.matmul(
                                    out=ph[:, 512 * qi + n0 : 512 * qi + n1],
                                    lhsT=ys[0:kp, j, 128 * b : 128 * b + 128],
                                    rhs=mats_h[t][0:kp, 0 : n1 - n0],
                                    start=(t == 0),
                                    stop=(t == 4),
                                )
                        nc.scalar.copy(
                            ot[:, bi, 1024 * qp : 1024 * qp + 1024], ph[:, :]
                        )
                if c == C - 1 and b2 == 1:
                    # last output: split per h-block so the first half's DMA
                    # overlaps the second half's evacuation
                    nc.sync.dma_start(y[c, b2, :, 0:1, :], ot[:, 0:1, :])
                    nc.sync.dma_start(y[c, b2, :, 1:2, :], ot[:, 1:2, :])
                else:
                    nc.sync.dma_start(y[c, b2], ot[:])
    return nc


def _get_nc():
    if "nc" not in _NC_CACHE:
        _NC_CACHE["nc"] = _build_nc()
    return _NC_CACHE["nc"]


def _shard_inputs(img):
    """img [1,3,4096,4096] f32 -> per-core packed fp16 slabs.

    x1 [C,128,4,PAD_W]: x1[c,p,t,:] = padded row 128*t+p of the slab (one
    contiguous 33 KB DMA descriptor per partition).  x2 [C,24,PAD_W]: the
    24 tail rows."""
    x = np.asarray(img)[0]
    xp = np.pad(
        x.astype(np.float16), ((0, 0), (HALF, HALF), (HALF, HALF)), mode="edge"
    )
    in_maps = []
    for core in range(N_CORES):
        sl = xp[:, SLAB * core : SLAB * core + IN_ROWS]      # [3, 536, 4120]
        tiles = np.zeros((C, 128, 5, PAD_W), np.float16)
        for t in range(5):
            rows = sl[:, 104 * t : min(104 * t + 128, IN_ROWS)]
            tiles[:, 0 : rows.shape[1], t, :] = rows
        x1a = np.ascontiguousarray(tiles[:, :, :, 0:2176])
        x1b = np.ascontiguousarray(tiles[:, :, :, 2176:PAD_W])
        in_maps.append({"x1a": x1a, "x1b": x1b})
    return in_maps


def kernel(img):
    from concourse.bass_utils import run_bass_kernel_spmd

    nc = _get_nc()
    in_maps = _shard_inputs(img)
    core_ids = list(range(N_CORES))

    import os

    trace = bool(os.environ.get("KNN_TRACE"))
    res = run_bass_kernel_spmd(nc, in_maps, core_ids, trace=trace)
    _NC_CACHE["last_exec_time_ns"] = res.exec_time_ns
    _NC_CACHE["last_results"] = res

    out = np.empty((C, H, W), np.float32)
    for core in core_ids:
        yc = res.results[core]["y"]                      # [C, 2, 128, 2, W]
        yc = yc.transpose(0, 1, 3, 2, 4).reshape(C, SLAB, W)
        out[:, SLAB * core : SLAB * (core + 1), :] = yc.astype(np.float32)
    return out


if __name__ == "__main__":
    # native compile smoke (no hardware)
    import tempfile
    from concourse.bass_utils import compile_bass_kernel

    nc = _build_nc()
    with tempfile.TemporaryDirectory() as td:
        neff = compile_bass_kernel(nc, td)
        print("COMPILED OK:", neff)
